# revision 36
# baseline (speedup 1.0000x reference)
"""Causal self-attention with RoPE — Trainium2 Bass/Tile kernel.

Problem: B=2, T=2048, C=2048, H=16 heads, D=128 head dim.
    qkv = x @ w_qkv ; RoPE(q, k) ; causal softmax attention ; out = attn_out @ w_out

Sharding (8 cores): core c handles batch b = c//4 and the 4 heads
hg = c%4 (heads 4*hg .. 4*hg+3).  Each core computes
    partial_c = attn_bh(x[b]) @ w_out[rows of its heads]      (shape [T, C])
and the host all-reduces: out[b] = sum of the 4 partials of batch b.

Per-core pipeline (all matmuls bf16 inputs, fp32 PSUM accumulate):
  A1) q/k projection, transposed layout ([D, T]) + RoPE in place (DVE),
      heads interleaved (q_h0, k_h0, q_h1, ...) so early heads rope first.
  A2) v projection chunks, interleaved INTO phase B through the shared
      attn@v PSUM ring: keeps the PE stream continuous while ScalarE's exp
      backlog drains.
  B)  Flash-style causal attention, qc-outer / head-inner, software
      pipelined (scores(jp+1) issued before attn@v(jp)):
      scoresT[k,q] via matmul -> exp on ScalarE (PSUM->SBUF bf16, 1024 wide
      to amortize ~540ns/instr ACT overhead) -> tri-mask (DVE) -> per-pair
      key-lane sums (DVE) -> half-width all-ones row-sum matmul -> softmax
      denominator 1/den via bit-magic + one Newton step on DVE (the native
      RECIPROCAL costs 6.6ns/elem) -> normalized otn (bf16).
      attn@v / row-sum streams skip causally-dead query columns.
  D)  Out-projection -> partial [T, C] fp32, chunked DMA out.
"""

import sys

for _p in ("/opt/trn_rl_repo",):
    if _p not in sys.path:
        sys.path.insert(0, _p)

import numpy as np
import ml_dtypes

import concourse.bass as bass
import concourse.mybir as mybir
import concourse.tile as tile

BF = mybir.dt.bfloat16
FP = mybir.dt.float32

BF_NP = ml_dtypes.bfloat16

NUM_HEADS = 16
B, T_FULL, C_FULL = 2, 2048, 2048
D = 128
N_CORES = 8
HPC = 4  # heads per core

ROPE_THETA = 10000.0


def _split_multi_waits(nc):
    """This container's walrus supports only ONE sync-wait per instruction
    ("Too many sync wait commands").  Hoist all but one wait of every
    multi-wait instruction onto preceding EventSemaphore instructions
    executed by the same engine's sequencer (block order = program order per
    engine) — same semantics, codegen-legal."""
    import bass_rust

    skip = (mybir.InstEventSemaphore,)
    ctr = 0
    for fn in nc.m.functions:
        for blk in fn.blocks:
            new_insts = None
            for idx, inst in enumerate(blk.instructions):
                si = inst.sync_info
                if (
                    not isinstance(inst, skip)
                    and si is not None
                    and si.on_wait
                    and len(si.on_wait) > 1
                ):
                    if new_insts is None:
                        new_insts = list(blk.instructions[:idx])
                    # keep the first wait (the data-dep one, usually latest to
                    # resolve) on the instruction itself; hoist the rest.
                    for w in si.on_wait[1:]:
                        ev = mybir.InstEventSemaphore(
                            name=f"I-dmaw{ctr}", ins=[], outs=[]
                        )
                        ctr += 1
                        ev.sync_info = bass_rust.SyncInfo(
                            on_wait=[w], on_update=[]
                        )
                        ev.engine = inst.engine
                        new_insts.append(ev)
                    inst.sync_info = bass_rust.SyncInfo(
                        on_wait=[si.on_wait[0]], on_update=si.on_update or []
                    )
                    new_insts.append(inst)
                elif new_insts is not None:
                    new_insts.append(inst)
            if new_insts is not None:
                blk.instructions = new_insts


class Cfg:
    """Kernel geometry. Full-size by default; shrinkable for simulator tests."""

    def __init__(self, T=T_FULL, C=C_FULL, hpc=HPC):
        assert T % 512 == 0 and C % 128 == 0
        self.T = T
        self.C = C
        self.hpc = hpc
        self.scale = 1.0 / np.sqrt(D)
        self.c_tiles = C // 128      # contraction tiles for QKV
        self.t_chunks = T // 512     # token chunks (QKV + queries)
        self.t_tiles = T // 128      # token tiles (keys / out rows)
        self.n_chunks = C // 512     # output-feature chunks for out-proj


def build_attention(cfg: Cfg):
    """Build the SPMD Bass program (identical on all cores; data differs)."""
    nc = bass.Bass("TRN2", debug=False, enable_partition_id=False)
    T, C, hpc = cfg.T, cfg.C, cfg.hpc
    F = hpc * D  # per-core q (or k, or v) feature count

    xT = nc.dram_tensor("xT", [C, T], BF, kind="ExternalInput")
    # wqk pre-packed per output-feature tile: [ft, p, (cc f)] so one 2D DMA
    # fetches one ft's full [C-chunk=128, C] weight tile.
    wqk = nc.dram_tensor("wqk", [2 * hpc * 128, C], BF, kind="ExternalInput")
    wv = nc.dram_tensor("wv", [C, F], BF, kind="ExternalInput")
    wout = nc.dram_tensor("wout", [F, C], BF, kind="ExternalInput")
    cosT = nc.dram_tensor("cosT", [D, T], BF, kind="ExternalInput")
    sinT = nc.dram_tensor("sinT", [D, T], BF, kind="ExternalInput")  # sign-baked
    masks = nc.dram_tensor("masks", [128, 4 * 512], BF, kind="ExternalInput")
    ones = nc.dram_tensor("ones", [128, 128], BF, kind="ExternalInput")
    ident = nc.dram_tensor("ident", [128, 128], BF, kind="ExternalInput")
    out = nc.dram_tensor("out", [T, C], FP, kind="ExternalOutput")

    Exp = mybir.ActivationFunctionType.Exp
    Ln = mybir.ActivationFunctionType.Ln

    with tile.TileContext(nc) as tc:
        # process q/k feature tiles interleaved per head (q_h0, k_h0, q_h1,
        # ...) so each head's RoPE finishes as early as possible — phase B's
        # first head groups start while later heads still rope
        FT_ORDER = [x for h in range(hpc) for x in (h, hpc + h)]

        with (
            tc.tile_pool(name="consts", bufs=1) as consts,
            tc.tile_pool(name="persist", bufs=1) as persist,
            tc.tile_pool(name="otp", bufs=1) as otp,
            tc.tile_pool(name="wo_pool", bufs=1) as wo_pool,
            tc.tile_pool(name="wv_pool", bufs=1) as wv_pool,
            tc.tile_pool(name="xpool", bufs=2) as xpool,
            tc.tile_pool(name="rope_tmp", bufs=1) as rope_tmp,
            tc.tile_pool(name="osb_pool", bufs=4) as osb_pool,
        ):
            wv_sb = [
                wv_pool.tile([128, F], BF, name=f"wv_sb{cc}", tag=f"wv{cc}")
                for cc in range(cfg.c_tiles)
            ]
            cos_sb = consts.tile([D, T], BF, name="cos_sb")
            sin_sb = consts.tile([D, T], BF, name="sin_sb")
            masks_sb = consts.tile([128, 4 * 512], BF, name="masks_sb")
            ones_sb = consts.tile([128, 128], BF, name="ones_sb")
            # seed constant for Newton reciprocal (exponent-flip magic)
            rcp_magic = consts.tile([128, 512], mybir.dt.int32, name="rcp_magic")
            nc.vector.memset(rcp_magic, 0x7EF311C3)

            def emit_late_const_dmas():
                # emitted AFTER the q/k weight DMAs: those gate the first
                # matmul, these aren't needed until later
                for cc in range(cfg.c_tiles):
                    nc.scalar.dma_start(
                        out=wv_sb[cc], in_=wv[cc * 128 : (cc + 1) * 128, :]
                    )
                nc.scalar.dma_start(out=cos_sb, in_=cosT[:, :])
                nc.scalar.dma_start(out=sin_sb, in_=sinT[:, :])
                nc.scalar.dma_start(out=masks_sb, in_=masks[:, :])
                nc.scalar.dma_start(out=ones_sb, in_=ones[:, :])

            # q/k transposed [D, T] per head (RoPE applied in place later);
            # v natural [T, F] stored as [128, t_tiles, F].
            qk_t = [
                persist.tile([D, T], BF, name=f"qk_t{ft}", tag=f"qk_t{ft}")
                for ft in range(2 * hpc)
            ]
            v_sb = persist.tile([128, cfg.t_tiles, F], BF, name="v_sb")

            def rope_inplace(ft, rope_tmp):
                # partition-swap copies on the (idle) GpSimd engine; muls and
                # add on DVE — shortens the rope tail that gates phase B
                t_cos = rope_tmp.tile([D, T], BF, tag="t_cos")
                nc.vector.tensor_mul(t_cos, qk_t[ft], cos_sb)
                t_shift = rope_tmp.tile([D, T], BF, tag="t_shift")
                nc.vector.tensor_copy(t_shift[0:64, :], qk_t[ft][64:128, :])
                nc.vector.tensor_copy(t_shift[64:128, :], qk_t[ft][0:64, :])
                nc.vector.tensor_mul(t_shift, t_shift, sin_sb)
                nc.vector.tensor_add(qk_t[ft], t_cos, t_shift)

            def load_x_chunk(tci):
                x_ch = []
                for cc in range(cfg.c_tiles):
                    x_t = xpool.tile([128, 512], BF, tag=f"x{cc}")
                    nc.sync.dma_start(
                        out=x_t,
                        in_=xT[
                            cc * 128 : (cc + 1) * 128,
                            tci * 512 : (tci + 1) * 512,
                        ],
                    )
                    x_ch.append(x_t)
                return x_ch

            # ---------------- Phase A1: q/k projection + RoPE ----------------
            with (
                tc.tile_pool(name="wqk_pool", bufs=1) as wqk_pool,
                tc.tile_pool(name="qkv_ps", bufs=3, space="PSUM") as qkv_ps,
            ):
                wqkf_sb = {}
                for ft in FT_ORDER:
                    wqkf_sb[ft] = wqk_pool.tile(
                        [128, C], BF, name=f"wqkf_sb{ft}", tag=f"wqk{ft}"
                    )
                # scalar-ring order: first weight tile (split small), the
                # first x chunk's odd tiles, remaining weights, then wv and
                # constants — matching first-use order on the PE
                ft0 = FT_ORDER[0]
                pw = C // 4
                for piece in range(4):
                    nc.scalar.dma_start(
                        out=wqkf_sb[ft0][:, piece * pw : (piece + 1) * pw],
                        in_=wqk[
                            ft0 * 128 : (ft0 + 1) * 128,
                            piece * pw : (piece + 1) * pw,
                        ],
                    )
                x0_cache = load_x_chunk(0)
                for ft in FT_ORDER[1:]:
                    nc.scalar.dma_start(
                        out=wqkf_sb[ft],
                        in_=wqk[ft * 128 : (ft + 1) * 128, :],
                    )
                emit_late_const_dmas()

                # A1: q/k (transposed layout), RoPE as soon as each row done
                for tci in range(cfg.t_chunks):
                    x_ch = x0_cache if tci == 0 else load_x_chunk(tci)
                    for ft in FT_ORDER:
                        ps_qk = qkv_ps.tile([128, 512], FP, tag="ps_qk")
                        for cc in range(cfg.c_tiles):
                            nc.tensor.matmul(
                                ps_qk,
                                lhsT=wqkf_sb[ft][:, cc * 128 : (cc + 1) * 128],
                                rhs=x_ch[cc],
                                start=(cc == 0),
                                stop=(cc == cfg.c_tiles - 1),
                            )
                        nc.vector.tensor_copy(
                            qk_t[ft][:, tci * 512 : (tci + 1) * 512], ps_qk
                        )
                        if tci == cfg.t_chunks - 1:
                            rope_inplace(ft, rope_tmp)

            # wout loads now so they land during phase B
            wout_sb = [
                wo_pool.tile([128, C], BF, name=f"wout_sb{h}", tag=f"wo{h}")
                for h in range(hpc)
            ]
            for h in range(hpc):
                nc.scalar.dma_start(
                    out=wout_sb[h], in_=wout[h * 128 : (h + 1) * 128, :]
                )

            # ---------------- Phase B: causal attention ----------------
            # scoresT blocks [k, q] so attn@v needs no transposes; exp runs
            # 1024-wide over PAIRS of 128-key tiles; row-sums accumulate via
            # an all-ones stationary matmul into the same PSUM tile as attn.
            # Software-pipelined: the PE issues scores(jp+1) before attn@v(jp)
            # so it never waits on the ScalarE exp latency.
            otn = [[None] * cfg.t_chunks for _ in range(hpc)]
            with (
                tc.tile_pool(name="expp", bufs=4) as expp,
                tc.tile_pool(name="psum_p", bufs=4) as psum_p,
                tc.tile_pool(name="rsp", bufs=2) as rsp,
                tc.tile_pool(name="sc_ps", bufs=2, space="PSUM") as sc_ps,
                tc.tile_pool(name="av_ps", bufs=2, space="PSUM") as av_ps,
                tc.tile_pool(name="bc_ps", bufs=2, space="PSUM") as bc_ps,
            ):
                pending_tail = []

                def flush_tail():
                    while pending_tail:
                        pending_tail.pop(0)()

                def a2_chunk(tci):
                    # v projection chunk, interleaved into phase B: shares
                    # the attn@v PSUM ring, so the PE stream stays continuous
                    # while the ScalarE exp backlog drains
                    x_ch = load_x_chunk(tci)
                    for tt in range(4):
                        ps_v = av_ps.tile([128, 512], FP, tag="ps_av")
                        for cc in range(cfg.c_tiles):
                            nc.tensor.matmul(
                                ps_v[:, 0:F],
                                lhsT=x_ch[cc][:, tt * 128 : (tt + 1) * 128],
                                rhs=wv_sb[cc],
                                start=(cc == 0),
                                stop=(cc == cfg.c_tiles - 1),
                            )
                        nc.scalar.copy(v_sb[:, tci * 4 + tt, :], ps_v[:, 0:F])

                def emit_group(h, qc):
                    q_h = qk_t[h]
                    k_h = qk_t[hpc + h]
                    if True:
                        nkp = (qc + 1) * 2  # causal: pairs of 128-key tiles
                        q_sl = q_h[:, qc * 512 : (qc + 1) * 512]
                        ps_av = av_ps.tile([128, 512], FP, tag="ps_av")
                        ps_bc = bc_ps.tile([128, 512], FP, tag="ps_bc")
                        exp_tiles = [None] * nkp
                        sum_tiles = [None] * nkp

                        # valid query range of key tile j starts at
                        # j*128 - qc*512 (clamped); columns below that are
                        # causally dead and skipped in attn@v / row-sum
                        # streams (exp+mask stay full-width so every byte
                        # read downstream is freshly written)
                        def rq(j):
                            return max(0, j * 128 - qc * 512)

                        def emit_sc_exp(
                            jp, nkp=nkp, exp_tiles=exp_tiles,
                            sum_tiles=sum_tiles, k_h=k_h, q_sl=q_sl,
                        ):
                            j0, j1 = 2 * jp, 2 * jp + 1
                            ps_sc = sc_ps.tile([128, 1024], FP, tag="ps_sc")
                            for half, j in ((0, j0), (1, j1)):
                                nc.tensor.matmul(
                                    ps_sc[:, half * 512 : (half + 1) * 512],
                                    lhsT=k_h[:, j * 128 : (j + 1) * 128],
                                    rhs=q_sl,
                                    start=True,
                                    stop=True,
                                )
                            expT = expp.tile([128, 1024], BF, tag="expT")
                            nc.scalar.activation(
                                expT, ps_sc, Exp, scale=float(cfg.scale)
                            )
                            if jp >= nkp - 2:
                                mi = jp - (nkp - 2)
                                nc.vector.tensor_mul(
                                    expT,
                                    expT,
                                    masks_sb[:, mi * 1024 : (mi + 1) * 1024],
                                )
                            # per-pair key-lane sums feed the (half-width)
                            # all-ones row-sum matmul: halves the PE cost of
                            # denominator accumulation
                            psum = psum_p.tile([128, 512], BF, tag="psum")
                            nc.vector.tensor_add(
                                psum, expT[:, 0:512], expT[:, 512:1024]
                            )
                            exp_tiles[jp] = expT
                            sum_tiles[jp] = psum

                        def emit_av(
                            jp, nkp=nkp, exp_tiles=exp_tiles,
                            sum_tiles=sum_tiles, ps_av=ps_av, ps_bc=ps_bc,
                            h=h, rq=rq,
                        ):
                            j0, j1 = 2 * jp, 2 * jp + 1
                            expT = exp_tiles[jp]
                            first, last = (jp == 0), (jp == nkp - 1)
                            r0, r1 = rq(j0), rq(j1)
                            nc.tensor.matmul(
                                ps_av[:, r0:512],
                                lhsT=v_sb[:, j0, h * 128 : (h + 1) * 128],
                                rhs=expT[:, r0:512],
                                start=first,
                                stop=False,
                            )
                            nc.tensor.matmul(
                                ps_av[:, r1:512],
                                lhsT=v_sb[:, j1, h * 128 : (h + 1) * 128],
                                rhs=expT[:, 512 + r1 : 1024],
                                start=False,
                                stop=last,
                            )
                            nc.tensor.matmul(
                                ps_bc[:, r0:512],
                                lhsT=ones_sb,
                                rhs=sum_tiles[jp][:, r0:512],
                                start=first,
                                stop=last,
                            )

                        def emit_tail(ps_av=ps_av, ps_bc=ps_bc, h=h, qc=qc):
                            # 1/den via bit-magic seed + one Newton step, all
                            # cheap DVE ops (InstReciprocal costs 6.6ns/elem;
                            # this costs 4 ops at ~1ns/elem and avoids any
                            # 2-PSUM-operand reads)
                            x0 = rsp.tile([128, 512], FP, tag="x0")
                            nc.vector.tensor_tensor(
                                out=x0.bitcast(mybir.dt.int32),
                                in0=rcp_magic,
                                in1=ps_bc.bitcast(mybir.dt.int32),
                                op=mybir.AluOpType.subtract,
                            )
                            t = rsp.tile([128, 512], FP, tag="t")
                            nc.vector.tensor_mul(t, ps_bc, x0)
                            y = rsp.tile([128, 512], FP, tag="y")
                            nc.vector.scalar_tensor_tensor(
                                out=y, in0=t, scalar=2.0, in1=x0,
                                op0=mybir.AluOpType.subtract,
                                op1=mybir.AluOpType.mult,
                            )  # y = (den*x0 - 2)*x0 = -recip
                            o = otp.tile(
                                [128, 512],
                                BF,
                                name=f"otn{h}_{qc}",
                                tag=f"otn{h}_{qc}",
                            )
                            nc.vector.scalar_tensor_tensor(
                                out=o, in0=ps_av, scalar=-1.0, in1=y,
                                op0=mybir.AluOpType.mult,
                                op1=mybir.AluOpType.mult,
                            )  # o = (-ps_av)*(-recip) = ps_av/den
                            otn[h][qc] = o

                        emit_sc_exp(0)
                        emit_sc_exp(1)
                        flush_tail()
                        for jp in range(2, nkp):
                            emit_sc_exp(jp)
                            emit_av(jp - 2)
                        emit_av(nkp - 2)
                        emit_av(nkp - 1)
                        pending_tail.append(emit_tail)

                # driver: v-projection chunks feed the qc-outer group sweep
                # two chunks ahead (rope tail + v availability cushion)
                a2_chunk(0)
                if cfg.t_chunks > 1:
                    a2_chunk(1)
                for qc in range(cfg.t_chunks):
                    if qc + 2 < cfg.t_chunks:
                        flush_tail()
                        a2_chunk(qc + 2)
                    for hi in range(hpc):
                        emit_group(hi, qc)
                flush_tail()

            # ---------------- Phase D: out-projection ----------------
            with (
                tc.tile_pool(name="o_ps", bufs=4, space="PSUM") as o_ps,
            ):
                for tt in range(cfg.t_tiles):
                    qc, off = tt // 4, (tt % 4) * 128
                    for n in range(cfg.n_chunks):
                        ps_o = o_ps.tile([128, 512], FP, tag="ps_o")
                        for h in range(hpc):
                            nc.tensor.matmul(
                                ps_o,
                                lhsT=otn[h][qc][:, off : off + 128],
                                rhs=wout_sb[h][:, n * 512 : (n + 1) * 512],
                                start=(h == 0),
                                stop=(h == hpc - 1),
                            )
                        osb = osb_pool.tile([128, 512], FP, tag="osb")
                        nc.scalar.copy(osb, ps_o)
                        # chunked output DMA: don't make the kernel tail wait
                        # for a whole [128, C] row to assemble
                        nc.sync.dma_start(
                            out=out[
                                tt * 128 : (tt + 1) * 128,
                                n * 512 : (n + 1) * 512,
                            ],
                            in_=osb,
                        )


    return nc


def rope_tables(T, dtype=np.float32):
    inv_freq = 1.0 / (ROPE_THETA ** (np.arange(0, D, 2, dtype=np.float32) / D))
    t = np.arange(T, dtype=np.float32)
    freqs = np.outer(t, inv_freq)  # [T, D/2]
    emb = np.concatenate([freqs, freqs], axis=-1)  # [T, D]
    return np.cos(emb).astype(dtype), np.sin(emb).astype(dtype)


def make_core_inputs(cfg: Cfg, x_b, w_qkv, w_out, cos, sin, hg):
    """Per-core input dict. x_b [T, C] fp32; w_qkv [C, 3C']; w_out [C', C];
    cos/sin [T, D]; hg = head-group index within the batch group."""
    T, C, hpc = cfg.T, cfg.C, cfg.hpc
    F = hpc * D
    H = w_qkv.shape[1] // 3 // D  # total heads in this (possibly shrunk) problem
    CQ = H * D

    f0 = hg * F
    xT = np.ascontiguousarray(x_b.T).astype(BF_NP)
    wq = w_qkv[:, f0 : f0 + F]
    wk = w_qkv[:, CQ + f0 : CQ + f0 + F]
    W = np.concatenate([wq, wk], axis=1)  # [C, 2F]
    # pack per-ft: wqk[ft*128+p, cc*128+f] = W[cc*128+p, ft*128+f]
    nft, ncc = 2 * hpc, C // 128
    wqk = np.ascontiguousarray(
        W.reshape(ncc, 128, nft, 128).transpose(2, 1, 0, 3).reshape(
            nft * 128, ncc * 128
        )
    ).astype(BF_NP)
    wv = np.ascontiguousarray(w_qkv[:, 2 * CQ + f0 : 2 * CQ + f0 + F]).astype(BF_NP)
    wout = np.ascontiguousarray(w_out[f0 : f0 + F, :]).astype(BF_NP)

    cosT = np.ascontiguousarray(cos.T).astype(BF_NP)  # [D, T]
    sinT = np.ascontiguousarray(sin.T).astype(np.float32)
    sinT[0:64, :] *= -1.0  # bake rotate_half sign
    sinT = sinT.astype(BF_NP)

    # diagonal-group masks: mask[mi][k, q] = 1 iff mi*128 + k <= q
    k_idx = np.arange(128)[:, None]
    q_idx = np.arange(512)[None, :]
    m = np.concatenate(
        [(mi * 128 + k_idx <= q_idx) for mi in range(4)], axis=1
    ).astype(BF_NP)

    return {
        "xT": xT,
        "wqk": wqk,
        "wv": wv,
        "wout": wout,
        "cosT": cosT,
        "sinT": sinT,
        "masks": np.ascontiguousarray(m),
        "ones": np.ones((128, 128), dtype=BF_NP),
        "ident": np.eye(128, dtype=BF_NP),
    }


_NC_CACHE = {}


def _get_nc(cfg: Cfg):
    key = (cfg.T, cfg.C, cfg.hpc)
    if key not in _NC_CACHE:
        nc = build_attention(cfg)
        _split_multi_waits(nc)  # HW codegen needs ≤1 wait per instruction
        _NC_CACHE[key] = nc
    return _NC_CACHE[key]


def kernel(x, cos, sin, w_qkv, w_out, trace=False, tmpdir=None):
    """Full-problem entry point: full inputs in, full [B, T, C] output back."""
    from concourse.bass_utils import run_bass_kernel_spmd

    x = np.asarray(x, dtype=np.float32)
    cos = np.asarray(cos, dtype=np.float32)
    sin = np.asarray(sin, dtype=np.float32)
    w_qkv = np.asarray(w_qkv, dtype=np.float32)
    w_out = np.asarray(w_out, dtype=np.float32)

    cfg = Cfg()
    nc = _get_nc(cfg)

    in_maps = []
    for c in range(N_CORES):
        b, hg = c // 4, c % 4
        in_maps.append(
            make_core_inputs(cfg, x[b], w_qkv, w_out, cos, sin, hg)
        )

    res = run_bass_kernel_spmd(
        nc,
        in_maps,
        core_ids=list(range(N_CORES)),
        trace=trace,
        tmpdir=tmpdir,
    )
    partials = [r["out"] for r in res.results]
    out = np.empty((B, cfg.T, cfg.C), dtype=np.float32)
    for b in range(B):
        out[b] = partials[4 * b] + partials[4 * b + 1]
        out[b] += partials[4 * b + 2]
        out[b] += partials[4 * b + 3]
    if trace:
        return out, res
    return out

